# revision 1
# baseline (speedup 1.0000x reference)
"""Contrastive (NT-Xent) loss kernel for 8 Trainium2 NeuronCores.

Math (matches the jax reference):
    z = l2_normalize(x, dim=1) for x = concat(emb_i, emb_j)   -> [8192, 256]
    sim = -(sq[a] + sq[b] - 2 z_a.z_b);  sq ~= 1  =>  sim/T = 20*dot - 20
    denom[a] = sum_{b != a} exp(sim[a,b]/T)
    pos[a]   = dot(z_a, z_partner(a)),  partner(a) = a +- 4096
    loss = mean_a( -(20*pos[a] - 20) + log(denom[a]) )

Distribution: 1024 rows of the sim matrix per core. Each core normalizes
its own 1024 rows, all-gathers the bf16 transposed representations, and
computes its row-slab with the gathered columns *rotated* by its own rank
(dynamic-offset DMAs driven by partition_id) so the diagonal block and the
positive-pair block sit at rank-independent column positions. While the
all-gather is in flight, "phase A" computes the slab columns that only
need local data (rotated columns [0, 1024)). Per-core outputs are the
1024 row denominators and positive dots; final log/sum runs on host.
"""

import sys

sys.path.insert(0, "/opt/trn_rl_repo")

import numpy as np

import concourse.bass as bass
import concourse.bacc as bacc
import concourse.mybir as mybir
import concourse.tile as tile
from concourse.bass_utils import run_bass_kernel_spmd

B = 4096
D = 256
N = 2 * B            # 8192 rows of the similarity matrix
NCORES = 8
LR = N // NCORES     # 1024 local rows per core
MT = LR // 128       # 8 m-tiles of 128 rows
TEMP = 0.1
SCALE = 2.0 / TEMP   # 20.0
BIAS = -2.0 / TEMP   # -20.0
NEG_BIG = -1.0e5     # added on the diagonal before exp -> exp() == 0

# phase B chunks cover rotated columns [1024, 8192)
B_CHUNKS = [(1024, 2048), (3072, 2048), (5120, 2048), (7168, 1024)]
NPARTS = 1 + len(B_CHUNKS)   # dsum columns per m-tile (phase A + B chunks)

f32 = mybir.dt.float32
bf16 = mybir.dt.bfloat16

_CACHE = {}


def build(loop_n: int | None = None):
    key = ("nc", loop_n)
    if key in _CACHE:
        return _CACHE[key]

    nc = bacc.Bacc(
        "TRN2", target_bir_lowering=False, debug=False, num_devices=NCORES
    )

    x_local = nc.declare_dram_parameter("x_local", [LR, D], f32, isOutput=False)
    ident_in = nc.declare_dram_parameter("ident", [128, 128], bf16, isOutput=False)
    eye_in = nc.declare_dram_parameter("eye", [128, 128], f32, isOutput=False)
    negeye_in = nc.declare_dram_parameter("negeye", [128, 128], f32, isOutput=False)
    denom_out = nc.declare_dram_parameter("denom_out", [128, MT], f32, isOutput=True)
    pos_out = nc.declare_dram_parameter("pos_out", [128, MT], f32, isOutput=True)

    with tile.TileContext(nc) as tc:
        with tc.tile_pool(name="sb", bufs=1) as sb, \
             tc.tile_pool(name="work", bufs=2) as work, \
             tc.tile_pool(name="dram", bufs=1, space="DRAM") as dram:

            ident = sb.tile([128, 128], bf16)
            nc.sync.dma_start(ident[:], ident_in[:])
            eye = sb.tile([128, 128], f32)
            nc.sync.dma_start(eye[:], eye_in[:])
            negeye = sb.tile([128, 128], f32)
            nc.sync.dma_start(negeye[:], negeye_in[:])

            bias_zero = sb.tile([128, 1], f32)
            nc.vector.memset(bias_zero[:], 0.0)
            bias_exp = sb.tile([128, 1], f32)
            nc.vector.memset(bias_exp[:], BIAS)

            # ---- normalize the local 1024 rows -------------------------
            x_all = sb.tile([128, MT * D], f32)
            for ti in range(MT):
                nc.sync.dma_start(
                    x_all[:, ti * D:(ti + 1) * D],
                    x_local[ti * 128:(ti + 1) * 128, :],
                )

            # Per-tile normalize pipeline, all transcendentals on ACT within
            # one table set (natural_log_exp_and_others holds square, ln,
            # exp, copy): square+rowsum via accum_out, 1/sqrt as
            # exp(-0.5*ln). No cross-tile barrier: tile 0 transposes while
            # tile 7 still loads.
            n2 = sb.tile([128, MT], f32)
            lnn = sb.tile([128, MT], f32)
            rinv = sb.tile([128, MT], f32)
            # zT_loc holds the transposed normalized local rows:
            # cols [0,1024) = k-half 0, cols [1024,2048) = k-half 1.
            zT_loc = sb.tile([128, 2 * LR], bf16)
            with tc.tile_pool(name="pre_psum", bufs=2, space="PSUM") as pps:
                for ti in range(MT):
                    xsq = work.tile([128, D], f32, tag="xsq")
                    nc.scalar.activation(
                        xsq[:], x_all[:, ti * D:(ti + 1) * D],
                        mybir.ActivationFunctionType.Square,
                        bias=bias_zero[:],
                        accum_out=n2[:, ti:ti + 1],
                    )
                    nc.scalar.activation(
                        lnn[:, ti:ti + 1], n2[:, ti:ti + 1],
                        mybir.ActivationFunctionType.Ln,
                        bias=bias_zero[:],
                    )
                    nc.scalar.activation(
                        rinv[:, ti:ti + 1], lnn[:, ti:ti + 1],
                        mybir.ActivationFunctionType.Exp,
                        bias=bias_zero[:], scale=-0.5,
                    )
                    z_t = work.tile([128, D], bf16, tag="z")
                    nc.vector.tensor_scalar_mul(
                        z_t[:], x_all[:, ti * D:(ti + 1) * D], rinv[:, ti:ti + 1]
                    )
                    pt = pps.tile([128, D], bf16)
                    nc.tensor.transpose(pt[:, 0:128], z_t[:, 0:128], ident[:])
                    nc.tensor.transpose(pt[:, 128:256], z_t[:, 128:256], ident[:])
                    nc.vector.tensor_copy(
                        zT_loc[:, ti * 128:(ti + 1) * 128], pt[:, 0:128]
                    )
                    nc.vector.tensor_copy(
                        zT_loc[:, LR + ti * 128:LR + (ti + 1) * 128],
                        pt[:, 128:256],
                    )

            # ---- all-gather the transposed reps, split in two halves so
            # the first starts while tiles 4-7 still normalize ------------
            HALF = LR // 2
            ag_in = [dram.tile([128, 2 * HALF], bf16, name=f"ag_in{h}")
                     for h in range(2)]
            ag_out = [dram.tile([NCORES, 128, 2 * HALF], bf16,
                                addr_space="Shared", name=f"ag_out{h}")
                      for h in range(2)]
            for h in range(2):
                for k in range(2):
                    nc.sync.dma_start(
                        ag_in[h][:, k * HALF:(k + 1) * HALF],
                        zT_loc[:, k * LR + h * HALF:k * LR + (h + 1) * HALF],
                    )
                nc.gpsimd.collective_compute(
                    "AllGather",
                    mybir.AluOpType.bypass,
                    ins=[ag_in[h].opt()],
                    outs=[ag_out[h].opt()],
                    replica_groups=[list(range(NCORES))],
                )

            den_parts = sb.tile([128, MT * NPARTS], f32)
            den_sb = sb.tile([128, MT], f32)
            pos_sb = sb.tile([128, MT], f32)

            with tc.tile_pool(name="mm_psum", bufs=2, space="PSUM") as mps:
                def phase_a():
                    # phase A: local columns [0,1024) — no AG dependency
                    for mi in range(MT):
                        ps = mps.tile([128, 2048], f32, tag="ps")
                        for k in range(2):
                            lhsT = zT_loc[:, k * LR + mi * 128:
                                          k * LR + (mi + 1) * 128]
                            for t in range(2):
                                nc.tensor.matmul(
                                    ps[:, t * 512:(t + 1) * 512],
                                    lhsT,
                                    zT_loc[:, k * LR + t * 512:
                                           k * LR + (t + 1) * 512],
                                    start=(k == 0),
                                    stop=(k == 1),
                                )
                        # rotated diagonal block: kill it before exp
                        nc.vector.tensor_tensor(
                            ps[:, mi * 128:(mi + 1) * 128],
                            ps[:, mi * 128:(mi + 1) * 128],
                            negeye[:],
                            op=mybir.AluOpType.add,
                        )
                        es = work.tile([128, 2048], f32, tag="es")
                        nc.scalar.activation(
                            es[:, 0:1024],
                            ps[:, 0:1024],
                            mybir.ActivationFunctionType.Exp,
                            bias=bias_exp[:],
                            scale=SCALE,
                            accum_out=den_parts[:, mi * NPARTS:
                                                mi * NPARTS + 1],
                        )

                zT = [sb.tile([128, N], bf16, name=f"zT_k{k}") for k in range(2)]

                def rotated_loads():
                    pid = nc.sync.partition_id()
                    for r in range(1, NCORES):
                        off = nc.sync.snap((pid + r) & (NCORES - 1), min_val=0,
                                           max_val=NCORES - 1)
                        for h in range(2):
                            for k in range(2):
                                nc.sync.dma_start(
                                    zT[k][:, r * LR + h * HALF:
                                          r * LR + (h + 1) * HALF],
                                    ag_out[h][bass.ds(off, 1), :,
                                              k * HALF:(k + 1) * HALF],
                                )

                def phase_b():
                    # phase B: gathered columns [1024, 8192)
                    for ci, (c0, cw) in enumerate(B_CHUNKS):
                        for mi in range(MT):
                            ps = mps.tile([128, 2048], f32, tag="ps")
                            for k in range(2):
                                lhsT = zT_loc[:, k * LR + mi * 128:
                                              k * LR + (mi + 1) * 128]
                                for t in range(cw // 512):
                                    col = c0 + t * 512
                                    nc.tensor.matmul(
                                        ps[:, t * 512:(t + 1) * 512],
                                        lhsT,
                                        zT[k][:, col:col + 512],
                                        start=(k == 0),
                                        stop=(k == 1),
                                    )
                            if c0 <= 4096 < c0 + cw:
                                # rotated positive block at col 4096+mi*128
                                o = 4096 - c0 + mi * 128
                                scr = work.tile([128, 128], f32, tag="scr")
                                nc.vector.tensor_mul(
                                    scr[:], ps[:, o:o + 128], eye[:]
                                )
                                nc.vector.reduce_sum(
                                    pos_sb[:, mi:mi + 1], scr[:],
                                    axis=mybir.AxisListType.X,
                                )
                            es = work.tile([128, 2048], f32, tag="es")
                            nc.scalar.activation(
                                es[:, 0:cw],
                                ps[:, 0:cw],
                                mybir.ActivationFunctionType.Exp,
                                bias=bias_exp[:],
                                scale=SCALE,
                                accum_out=den_parts[:, mi * NPARTS + 1 + ci:
                                                    mi * NPARTS + 2 + ci],
                            )

                if loop_n is None:
                    phase_a()
                    rotated_loads()
                    phase_b()
                else:
                    # timing variant: loads first, then HW-loop the compute
                    rotated_loads()
                    with tc.For_i(0, loop_n, 1):
                        phase_a()
                        phase_b()

                for mi in range(MT):
                    nc.vector.reduce_sum(
                        den_sb[:, mi:mi + 1],
                        den_parts[:, mi * NPARTS:(mi + 1) * NPARTS],
                        axis=mybir.AxisListType.X,
                    )

            nc.sync.dma_start(denom_out[:], den_sb[:])
            nc.sync.dma_start(pos_out[:], pos_sb[:])

    nc.compile()
    _CACHE[key] = nc
    return nc


def make_in_maps(emb_i: np.ndarray, emb_j: np.ndarray):
    x_full = np.ascontiguousarray(
        np.concatenate([np.asarray(emb_i), np.asarray(emb_j)], axis=0),
        dtype=np.float32,
    )
    bf16_np = mybir.dt.np(bf16)
    ident = np.eye(128, dtype=np.float32).astype(bf16_np)
    eye = np.eye(128, dtype=np.float32)
    negeye = (NEG_BIG * np.eye(128)).astype(np.float32)
    return [
        {
            "x_local": x_full[c * LR:(c + 1) * LR],
            "ident": ident,
            "eye": eye,
            "negeye": negeye,
        }
        for c in range(NCORES)
    ]


def finish_on_host(results) -> np.ndarray:
    total = 0.0
    for r in results:
        den = r["denom_out"].astype(np.float64)
        pos = r["pos_out"].astype(np.float64)
        total += np.sum(-(SCALE * pos + BIAS) + np.log(den))
    return np.array(total / N, dtype=np.float32)


def kernel(emb_i: np.ndarray, emb_j: np.ndarray) -> np.ndarray:
    nc = build()
    in_maps = make_in_maps(emb_i, emb_j)
    res = run_bass_kernel_spmd(nc, in_maps, core_ids=list(range(NCORES)))
    return finish_on_host(res.results)


if __name__ == "__main__":
    rng = np.random.default_rng(0)
    ei = rng.standard_normal((B, D), dtype=np.float32)
    ej = rng.standard_normal((B, D), dtype=np.float32)
    print(kernel(ei, ej))

